# revision 6
# baseline (speedup 1.0000x reference)
"""Full-width attention (B=4, S=2048, D=1024, no head split) on 8 TRN2 cores.

Sharding: data-parallel over (batch, query-half) -> 8 shards. Core c handles
batch b = c//2, query rows [h*1024, (h+1)*1024) with h = c%2. Each core
computes K/V projections for its full batch (redundantly with its pair core),
Q projection for its query half, then scores^T -> exp -> AV locally.

Layout trick: everything is computed without any on-device transposes.
  - host passes x^T (d-major) per batch, plus W^T for each projection
  - Q^T[e,s] = (Wq^T)^T.T @ x^T   (lhsT=WqT, rhs=xT)  -> e on partitions
  - K^T[e,s] likewise, staged to DRAM scratch and re-streamed
  - V[s,e]   = (x^T).T @ Wv^T     (lhsT=xT,  rhs=WvT) -> s on partitions
  - scores^T[k,q] = KT.T @ QT (contract e)            -> k on partitions
  - softmax without max-subtraction (|scores| <= ~25, exp is safe in fp32):
    E = exp(scores^T / 8); rowsum via matmul with ones-vector rhs;
    out[q,e] = E.T @ V (contract k), scaled by 1/rowsum per partition.
  - bv folded in at the end: softmax rows sum to 1, so out += bv.
All matmuls run as float32r (1-pass FP22) at full PE speed.
"""

import math
from contextlib import ExitStack

import numpy as np

P = 128
B, S, D = 4, 2048, 1024
SQ = 1024  # query rows per core
KO = D // P  # 8 chunks of contraction dim
N_CORES = 8
F32 = None  # set after mybir import


def _apply_drain_patch():
    # This container's walrus rejects >1 sync-wait on the kernel-tail Drain
    # ("Too many sync wait commands"). Split the global-clock waits across
    # multiple drain instructions, one wait each.
    from concourse import tile as tile_mod
    from concourse.vector_clock import VectorClock, ScopedClock

    if getattr(tile_mod.TileContext, "_drain_patched", False):
        return

    def _split_drain_and_barrier(self, tick_clock, wait_clock):
        gc = tick_clock.global_clock
        n = len(gc)
        procs = [i for i in range(n) if gc[i] > 0]
        if not procs:
            d = self.nc.sync.drain()
            wait_clock.add_sem_waits(d.ins, ScopedClock({None: gc}))
        for p in procs:
            vc = VectorClock([gc[i] if i == p else 0 for i in range(n)])
            d = self.nc.sync.drain()
            wait_clock.add_sem_waits(d.ins, ScopedClock({None: vc}))
        self.nc.all_engine_barrier()
        assert self.sems is not None
        popped = self.nc._tile_sem_poison_stack.pop()
        assert popped is self._sem_poison
        self.nc.clear_and_free_semaphores(list(self.sems.allocated().values()))
        self.nc.all_engine_barrier()

    tile_mod.TileContext._drain_and_barrier = _split_drain_and_barrier
    tile_mod.TileContext._drain_patched = True


def build_bass():
    from concourse import bacc
    import concourse.mybir as mybir
    from concourse.tile import TileContext

    f32 = mybir.dt.float32
    f32r = mybir.dt.float32r
    AF = mybir.ActivationFunctionType

    nc = bacc.Bacc(
        "TRN2",
        target_bir_lowering=False,
        debug=False,
        enable_asserts=False,
        num_devices=N_CORES,
    )

    xT = nc.dram_tensor("xT", [D, S], f32r, kind="ExternalInput")
    xTq = nc.dram_tensor("xTq", [D, SQ], f32r, kind="ExternalInput")
    wqT = nc.dram_tensor("wqT", [D, D], f32r, kind="ExternalInput")
    wkT = nc.dram_tensor("wkT", [D, D], f32r, kind="ExternalInput")
    wvT = nc.dram_tensor("wvT", [D, D], f32r, kind="ExternalInput")
    bqp = nc.dram_tensor("bqp", [P, KO], f32, kind="ExternalInput")
    bkp = nc.dram_tensor("bkp", [P, KO], f32, kind="ExternalInput")
    bvb = nc.dram_tensor("bvb", [P, D], f32, kind="ExternalInput")
    ones = nc.dram_tensor("ones", [P, 2], f32r, kind="ExternalInput")
    out = nc.dram_tensor("out", [SQ, D], f32, kind="ExternalOutput")

    xT_r = xT[:, :].rearrange("(ko p) s -> p ko s", p=P)
    xTq_r = xTq[:, :].rearrange("(ko p) s -> p ko s", p=P)
    wqT_r = wqT[:, :].rearrange("(ko p) e -> p ko e", p=P)
    wkT_r = wkT[:, :].rearrange("(ko p) e -> p ko e", p=P)
    wvT_r = wvT[:, :].rearrange("(ko p) e -> p ko e", p=P)

    with TileContext(nc) as tc, ExitStack() as ctx:
        qt_pool = ctx.enter_context(tc.tile_pool(name="qtp", bufs=1))
        v_pool = ctx.enter_context(tc.tile_pool(name="vp", bufs=1))
        cpool = ctx.enter_context(tc.tile_pool(name="cp", bufs=1))
        psA_p = ctx.enter_context(tc.tile_pool(name="psA", bufs=2, space="PSUM"))
        psB_p = ctx.enter_context(tc.tile_pool(name="psB", bufs=2, space="PSUM"))
        psC_p = ctx.enter_context(tc.tile_pool(name="psC", bufs=2, space="PSUM"))
        psR_p = ctx.enter_context(tc.tile_pool(name="psR", bufs=2, space="PSUM"))
        dram_p = ctx.enter_context(tc.tile_pool(name="drp", bufs=1, space="DRAM"))

        qt = qt_pool.tile([P, KO, SQ], f32r)  # Q^T, e on partitions
        v = v_pool.tile([P, S // P, D], f32r)  # V, s on partitions
        kt_dram = dram_p.tile([P, KO, S], f32r)  # K^T staging

        bqp_t = cpool.tile([P, KO], f32)
        nc.sync.dma_start(bqp_t[:], bqp[:, :])
        bkp_t = cpool.tile([P, KO], f32)
        nc.sync.dma_start(bkp_t[:], bkp[:, :])
        bvb_t = cpool.tile([P, D], f32)
        nc.sync.dma_start(bvb_t[:], bvb[:, :])
        ones_t = cpool.tile([P, 2], f32r)
        nc.sync.dma_start(ones_t[:], ones[:, :])

        # ---------------- Phase 1: projections ----------------
        with (
            tc.tile_pool(name="xtp", bufs=2) as xt_pool,
            tc.tile_pool(name="wp", bufs=3) as w_pool,
            tc.tile_pool(name="kst", bufs=3) as kst_pool,
        ):
            # --- V = x @ Wv^T ---
            wv = []
            for half in range(2):
                wt = w_pool.tile([P, KO, 512], f32r, tag="w", name=f"wv{half}")
                nc.sync.dma_start(wt[:], wvT_r[:, :, half * 512 : (half + 1) * 512])
                wv.append(wt)
            for sc in range(4):
                xt_c = xt_pool.tile([P, KO, 512], f32r, tag="xt", name=f"xtv{sc}")
                nc.sync.dma_start(xt_c[:], xT_r[:, :, sc * 512 : (sc + 1) * 512])
                for ss in range(4):
                    si = sc * 4 + ss
                    pb = psB_p.tile([P, 512], f32, tag="psB", name="pbv")
                    pc = psC_p.tile([P, 512], f32, tag="psC", name="pcv")
                    for ko in range(KO):
                        lh = xt_c[:, ko, ss * P : (ss + 1) * P]
                        nc.tensor.matmul(
                            pb[:], lh, wv[0][:, ko, :],
                            start=(ko == 0), stop=(ko == KO - 1),
                        )
                        nc.tensor.matmul(
                            pc[:], lh, wv[1][:, ko, :],
                            start=(ko == 0), stop=(ko == KO - 1),
                        )
                    nc.vector.tensor_copy(v[:, si, 0:512], pb[:])
                    nc.vector.tensor_copy(v[:, si, 512:1024], pc[:])

            # --- K^T = Wk @ x^T (+bk), staged to DRAM ---
            wk = []
            for half in range(2):
                wt = w_pool.tile([P, KO, 512], f32r, tag="w", name=f"wk{half}")
                nc.sync.dma_start(wt[:], wkT_r[:, :, half * 512 : (half + 1) * 512])
                wk.append(wt)
            for sc in range(4):
                xt_c = xt_pool.tile([P, KO, 512], f32r, tag="xt", name=f"xtk{sc}")
                nc.sync.dma_start(xt_c[:], xT_r[:, :, sc * 512 : (sc + 1) * 512])
                for eo in range(KO):
                    pa = psA_p.tile([P, 512], f32, tag="psA", name="pak")
                    wkh = wk[eo // 4]
                    col = (eo % 4) * P
                    for ko in range(KO):
                        nc.tensor.matmul(
                            pa[:], wkh[:, ko, col : col + P], xt_c[:, ko, :],
                            start=(ko == 0), stop=(ko == KO - 1),
                        )
                    kst = kst_pool.tile([P, 512], f32r, tag="kst", name="kst")
                    nc.vector.tensor_scalar_add(kst[:], pa[:], bkp_t[:, eo : eo + 1])
                    nc.sync.dma_start(kt_dram[:, eo, sc * 512 : (sc + 1) * 512], kst[:])

            # --- Q^T = Wq @ xq^T (+bq), kept resident ---
            wq = []
            for half in range(2):
                wt = w_pool.tile([P, KO, 512], f32r, tag="w", name=f"wq{half}")
                nc.sync.dma_start(wt[:], wqT_r[:, :, half * 512 : (half + 1) * 512])
                wq.append(wt)
            for qc2 in range(2):
                xt_c = xt_pool.tile([P, KO, 512], f32r, tag="xt", name=f"xtq{qc2}")
                nc.sync.dma_start(xt_c[:], xTq_r[:, :, qc2 * 512 : (qc2 + 1) * 512])
                for eo in range(KO):
                    pa = psA_p.tile([P, 512], f32, tag="psA", name="paq")
                    wqh = wq[eo // 4]
                    col = (eo % 4) * P
                    for ko in range(KO):
                        nc.tensor.matmul(
                            pa[:], wqh[:, ko, col : col + P], xt_c[:, ko, :],
                            start=(ko == 0), stop=(ko == KO - 1),
                        )
                    nc.vector.tensor_scalar_add(
                        qt[:, eo, qc2 * 512 : (qc2 + 1) * 512],
                        pa[:],
                        bqp_t[:, eo : eo + 1],
                    )

        # ---------------- Phase 2: attention ----------------
        with (
            tc.tile_pool(name="ep", bufs=1) as e_pool,
            tc.tile_pool(name="ktp", bufs=2) as kt_pool,
            tc.tile_pool(name="osp", bufs=3) as out_pool,
            tc.tile_pool(name="msc", bufs=4) as msc_pool,
        ):
            inv_sqrt_dk = 1.0 / math.sqrt(D // 16)  # d_key = 64
            for qc in range(2):
                E = e_pool.tile([P, S // P, 512], f32r, tag="E", name="E")
                q_sl = qt[:, :, qc * 512 : (qc + 1) * 512]
                for kc in range(4):
                    kt_in = kt_pool.tile([P, KO, 512], f32r, tag="ktin", name="ktin")
                    nc.sync.dma_start(kt_in[:], kt_dram[:, :, kc * 512 : (kc + 1) * 512])
                    for ks in range(4):
                        pa = psA_p.tile([P, 512], f32, tag="psA", name="pas")
                        for eo in range(KO):
                            nc.tensor.matmul(
                                pa[:],
                                kt_in[:, eo, ks * P : (ks + 1) * P],
                                q_sl[:, eo, :],
                                start=(eo == 0), stop=(eo == KO - 1),
                            )
                        nc.scalar.activation(
                            E[:, kc * 4 + ks, :], pa[:], AF.Exp, scale=inv_sqrt_dk
                        )
                for qs in range(4):
                    pb = psB_p.tile([P, 512], f32, tag="psB", name="pbav")
                    pc = psC_p.tile([P, 512], f32, tag="psC", name="pcav")
                    pr = psR_p.tile([P, 2], f32, tag="psR", name="prav")
                    for ko in range(S // P):
                        lh = E[:, ko, qs * P : (qs + 1) * P]
                        nc.tensor.matmul(
                            pb[:], lh, v[:, ko, 0:512],
                            start=(ko == 0), stop=(ko == S // P - 1),
                        )
                        nc.tensor.matmul(
                            pc[:], lh, v[:, ko, 512:1024],
                            start=(ko == 0), stop=(ko == S // P - 1),
                        )
                        nc.tensor.matmul(
                            pr[:], lh, ones_t[:],
                            start=(ko == 0), stop=(ko == S // P - 1),
                        )
                    recip = msc_pool.tile([P, 1], f32, tag="recip", name="recip")
                    nc.vector.reciprocal(recip[:], pr[:, 0:1])
                    row0 = qc * 512 + qs * P
                    for half, ps in ((0, pb), (1, pc)):
                        o = out_pool.tile([P, 512], f32, tag="ost", name="ost")
                        nc.vector.tensor_scalar_mul(o[:], ps[:], recip[:])
                        nc.vector.tensor_add(
                            o[:], o[:], bvb_t[:, half * 512 : (half + 1) * 512]
                        )
                        nc.sync.dma_start(
                            out[row0 : row0 + P, half * 512 : (half + 1) * 512], o[:]
                        )

    nc.finalize()
    return nc


def make_in_maps(x, Wq, bq, Wk, bk, Wv, bv):
    """Build the 8 per-core input maps from full inputs."""
    x = np.asarray(x, dtype=np.float32)
    wqT = np.ascontiguousarray(np.asarray(Wq, np.float32).T)
    wkT = np.ascontiguousarray(np.asarray(Wk, np.float32).T)
    wvT = np.ascontiguousarray(np.asarray(Wv, np.float32).T)
    bqp = np.ascontiguousarray(np.asarray(bq, np.float32).reshape(KO, P).T)
    bkp = np.ascontiguousarray(np.asarray(bk, np.float32).reshape(KO, P).T)
    bvb = np.ascontiguousarray(
        np.broadcast_to(np.asarray(bv, np.float32), (P, D))
    )
    ones_np = np.ones((P, 2), np.float32)
    xT_b = [np.ascontiguousarray(x[b].T) for b in range(B)]
    in_maps = []
    for c in range(N_CORES):
        b, h = c // 2, c % 2
        in_maps.append(
            {
                "xT": xT_b[b],
                "xTq": np.ascontiguousarray(x[b, h * SQ : (h + 1) * SQ].T),
                "wqT": wqT,
                "wkT": wkT,
                "wvT": wvT,
                "bqp": bqp,
                "bkp": bkp,
                "bvb": bvb,
                "ones": ones_np,
            }
        )
    return in_maps


_NC_CACHE = None


def get_nc():
    global _NC_CACHE
    if _NC_CACHE is None:
        _NC_CACHE = build_bass()
    return _NC_CACHE


def kernel(x, Wq, bq, Wk, bk, Wv, bv, **run_kwargs):
    from concourse.bass_utils import run_bass_kernel_spmd

    nc = get_nc()
    in_maps = make_in_maps(x, Wq, bq, Wk, bk, Wv, bv)
    res = run_bass_kernel_spmd(
        nc, in_maps, core_ids=list(range(N_CORES)), **run_kwargs
    )
    out = np.empty((B, S, D), dtype=np.float32)
    for c in range(N_CORES):
        b, h = c // 2, c % 2
        out[b, h * SQ : (h + 1) * SQ, :] = res.results[c]["out"]
    if run_kwargs.get("trace"):
        kernel.last_results = res
    return out


# revision 7
# speedup vs baseline: 1.0665x; 1.0665x over previous
"""Full-width attention (B=4, S=2048, D=1024, no head split) on 8 TRN2 cores.

Sharding: data-parallel over (batch, query-half) -> 8 shards. Core c handles
batch b = c//2, query rows [h*1024, (h+1)*1024) with h = c%2. Each core
computes K/V projections for its full batch (redundantly with its pair core),
Q projection for its query half, then scores^T -> exp -> AV locally.

Layout trick: everything is computed without any on-device transposes.
  - host passes x^T (d-major) per batch, plus W^T for each projection
  - Q^T[e,s] = (Wq^T)^T.T @ x^T   (lhsT=WqT, rhs=xT)  -> e on partitions
  - K^T[e,s] likewise, staged to DRAM scratch and re-streamed
  - V[s,e]   = (x^T).T @ Wv^T     (lhsT=xT,  rhs=WvT) -> s on partitions
  - scores^T[k,q] = KT.T @ QT (contract e)            -> k on partitions
  - softmax without max-subtraction (|scores| <= ~25, exp is safe in fp32):
    E = exp(scores^T / 8); rowsum via matmul with ones-vector rhs;
    out[q,e] = E.T @ V (contract k), scaled by 1/rowsum per partition.
  - bv folded in at the end: softmax rows sum to 1, so out += bv.
All matmuls run as float32r (1-pass FP22) at full PE speed.
"""

import math
from contextlib import ExitStack

import numpy as np

P = 128
B, S, D = 4, 2048, 1024
SQ = 1024  # query rows per core
KO = D // P  # 8 chunks of contraction dim
N_CORES = 8
F32 = None  # set after mybir import


def _apply_drain_patch():
    # This container's walrus rejects >1 sync-wait on the kernel-tail Drain
    # ("Too many sync wait commands"). Split the global-clock waits across
    # multiple drain instructions, one wait each.
    from concourse import tile as tile_mod
    from concourse.vector_clock import VectorClock, ScopedClock

    if getattr(tile_mod.TileContext, "_drain_patched", False):
        return

    def _split_drain_and_barrier(self, tick_clock, wait_clock):
        gc = tick_clock.global_clock
        n = len(gc)
        procs = [i for i in range(n) if gc[i] > 0]
        if not procs:
            d = self.nc.sync.drain()
            wait_clock.add_sem_waits(d.ins, ScopedClock({None: gc}))
        for p in procs:
            vc = VectorClock([gc[i] if i == p else 0 for i in range(n)])
            d = self.nc.sync.drain()
            wait_clock.add_sem_waits(d.ins, ScopedClock({None: vc}))
        self.nc.all_engine_barrier()
        assert self.sems is not None
        popped = self.nc._tile_sem_poison_stack.pop()
        assert popped is self._sem_poison
        self.nc.clear_and_free_semaphores(list(self.sems.allocated().values()))
        self.nc.all_engine_barrier()

    tile_mod.TileContext._drain_and_barrier = _split_drain_and_barrier
    tile_mod.TileContext._drain_patched = True


def build_bass():
    from concourse import bacc
    import concourse.mybir as mybir
    from concourse.tile import TileContext

    f32 = mybir.dt.float32
    f32r = mybir.dt.float32r
    AF = mybir.ActivationFunctionType

    nc = bacc.Bacc(
        "TRN2",
        target_bir_lowering=False,
        debug=False,
        enable_asserts=False,
        num_devices=N_CORES,
    )

    xT = nc.dram_tensor("xT", [D, S], f32r, kind="ExternalInput")
    xTq = nc.dram_tensor("xTq", [D, SQ], f32r, kind="ExternalInput")
    wqT = nc.dram_tensor("wqT", [D, D], f32r, kind="ExternalInput")
    wkT = nc.dram_tensor("wkT", [D, D], f32r, kind="ExternalInput")
    wvT = nc.dram_tensor("wvT", [D, D], f32r, kind="ExternalInput")
    bqp = nc.dram_tensor("bqp", [P, KO], f32, kind="ExternalInput")
    bkp = nc.dram_tensor("bkp", [P, KO], f32, kind="ExternalInput")
    bvb = nc.dram_tensor("bvb", [P, D], f32, kind="ExternalInput")
    ones = nc.dram_tensor("ones", [P, 2], f32r, kind="ExternalInput")
    out = nc.dram_tensor("out", [SQ, D], f32, kind="ExternalOutput")

    xT_r = xT[:, :].rearrange("(ko p) s -> p ko s", p=P)
    xTq_r = xTq[:, :].rearrange("(ko p) s -> p ko s", p=P)
    wqT_r = wqT[:, :].rearrange("(ko p) e -> p ko e", p=P)
    wkT_r = wkT[:, :].rearrange("(ko p) e -> p ko e", p=P)
    wvT_r = wvT[:, :].rearrange("(ko p) e -> p ko e", p=P)

    with TileContext(nc) as tc, ExitStack() as ctx:
        qt_pool = ctx.enter_context(tc.tile_pool(name="qtp", bufs=1))
        v_pool = ctx.enter_context(tc.tile_pool(name="vp", bufs=1))
        cpool = ctx.enter_context(tc.tile_pool(name="cp", bufs=1))
        psA_p = ctx.enter_context(tc.tile_pool(name="psA", bufs=2, space="PSUM"))
        psB_p = ctx.enter_context(tc.tile_pool(name="psB", bufs=2, space="PSUM"))
        psC_p = ctx.enter_context(tc.tile_pool(name="psC", bufs=2, space="PSUM"))
        psR_p = ctx.enter_context(tc.tile_pool(name="psR", bufs=2, space="PSUM"))
        dram_p = ctx.enter_context(tc.tile_pool(name="drp", bufs=1, space="DRAM"))

        qt = qt_pool.tile([P, KO, SQ], f32r)  # Q^T, e on partitions
        v = v_pool.tile([P, S // P, D], f32r)  # V, s on partitions
        kt_dram = dram_p.tile([P, KO, S], f32r)  # K^T staging

        bqp_t = cpool.tile([P, KO], f32)
        nc.gpsimd.dma_start(bqp_t[:], bqp[:, :])
        bkp_t = cpool.tile([P, KO], f32)
        nc.gpsimd.dma_start(bkp_t[:], bkp[:, :])
        bvb_t = cpool.tile([P, D], f32)
        nc.gpsimd.dma_start(bvb_t[:], bvb[:, :])
        ones_t = cpool.tile([P, 2], f32r)
        nc.gpsimd.dma_start(ones_t[:], ones[:, :])

        # ---------------- Phase 1: projections ----------------
        with (
            tc.tile_pool(name="xtp", bufs=3) as xt_pool,
            tc.tile_pool(name="wp", bufs=3) as w_pool,
            tc.tile_pool(name="kst", bufs=3) as kst_pool,
        ):
            # --- V = x @ Wv^T ---
            wv = [
                w_pool.tile([P, KO, 512], f32r, tag="w", name=f"wv{half}")
                for half in range(2)
            ]
            xtv0 = xt_pool.tile([P, KO, 512], f32r, tag="xt", name="xtv0")
            for ko in range(KO):
                nc.sync.dma_start(wv[0][:, ko, :], wvT_r[:, ko, 0:512])
                nc.sync.dma_start(wv[1][:, ko, :], wvT_r[:, ko, 512:1024])
                nc.sync.dma_start(xtv0[:, ko, :], xT_r[:, ko, 0:512])
            for sc in range(4):
                if sc == 0:
                    xt_c = xtv0
                else:
                    xt_c = xt_pool.tile([P, KO, 512], f32r, tag="xt", name=f"xtv{sc}")
                    for ko in range(KO):
                        nc.sync.dma_start(
                            xt_c[:, ko, :], xT_r[:, ko, sc * 512 : (sc + 1) * 512]
                        )
                for ss in range(4):
                    si = sc * 4 + ss
                    pb = psB_p.tile([P, 512], f32, tag="psB", name="pbv")
                    pc = psC_p.tile([P, 512], f32, tag="psC", name="pcv")
                    for ko in range(KO):
                        lh = xt_c[:, ko, ss * P : (ss + 1) * P]
                        nc.tensor.matmul(
                            pb[:], lh, wv[0][:, ko, :],
                            start=(ko == 0), stop=(ko == KO - 1),
                        )
                        nc.tensor.matmul(
                            pc[:], lh, wv[1][:, ko, :],
                            start=(ko == 0), stop=(ko == KO - 1),
                        )
                    nc.scalar.copy(v[:, si, 0:512], pb[:])
                    nc.scalar.copy(v[:, si, 512:1024], pc[:])

            # --- K^T = Wk @ x^T (+bk), staged to DRAM ---
            wk = []
            for half in range(2):
                wt = w_pool.tile([P, KO, 512], f32r, tag="w", name=f"wk{half}")
                nc.sync.dma_start(wt[:], wkT_r[:, :, half * 512 : (half + 1) * 512])
                wk.append(wt)
            for sc in range(4):
                xt_c = xt_pool.tile([P, KO, 512], f32r, tag="xt", name=f"xtk{sc}")
                for ko in range(KO):
                    nc.sync.dma_start(
                        xt_c[:, ko, :], xT_r[:, ko, sc * 512 : (sc + 1) * 512]
                    )
                for eo in range(KO):
                    pa = psA_p.tile([P, 512], f32, tag="psA", name="pak")
                    wkh = wk[eo // 4]
                    col = (eo % 4) * P
                    for ko in range(KO):
                        nc.tensor.matmul(
                            pa[:], wkh[:, ko, col : col + P], xt_c[:, ko, :],
                            start=(ko == 0), stop=(ko == KO - 1),
                        )
                    kst = kst_pool.tile([P, 512], f32r, tag="kst", name="kst")
                    nc.scalar.activation(kst[:], pa[:], AF.Identity, bias=bkp_t[:, eo : eo + 1])
                    nc.gpsimd.dma_start(kt_dram[:, eo, sc * 512 : (sc + 1) * 512], kst[:])

            # --- Q^T = Wq @ xq^T (+bq), kept resident ---
            wq = []
            for half in range(2):
                wt = w_pool.tile([P, KO, 512], f32r, tag="w", name=f"wq{half}")
                nc.sync.dma_start(wt[:], wqT_r[:, :, half * 512 : (half + 1) * 512])
                wq.append(wt)
            for qc2 in range(2):
                xt_c = xt_pool.tile([P, KO, 512], f32r, tag="xt", name=f"xtq{qc2}")
                for ko in range(KO):
                    nc.sync.dma_start(
                        xt_c[:, ko, :], xTq_r[:, ko, qc2 * 512 : (qc2 + 1) * 512]
                    )
                for eo in range(KO):
                    pa = psA_p.tile([P, 512], f32, tag="psA", name="paq")
                    wqh = wq[eo // 4]
                    col = (eo % 4) * P
                    for ko in range(KO):
                        nc.tensor.matmul(
                            pa[:], wqh[:, ko, col : col + P], xt_c[:, ko, :],
                            start=(ko == 0), stop=(ko == KO - 1),
                        )
                    nc.scalar.activation(
                        qt[:, eo, qc2 * 512 : (qc2 + 1) * 512],
                        pa[:],
                        AF.Identity,
                        bias=bqp_t[:, eo : eo + 1],
                    )

        # ---------------- Phase 2: attention ----------------
        with (
            tc.tile_pool(name="ep", bufs=1) as e_pool,
            tc.tile_pool(name="ktp", bufs=2) as kt_pool,
            tc.tile_pool(name="osp", bufs=3) as out_pool,
            tc.tile_pool(name="msc", bufs=4) as msc_pool,
        ):
            inv_sqrt_dk = 1.0 / math.sqrt(D // 16)  # d_key = 64
            for qc in range(2):
                E = e_pool.tile([P, S // P, 512], f32r, tag="E", name="E")
                q_sl = qt[:, :, qc * 512 : (qc + 1) * 512]
                for kc in range(4):
                    kt_in = kt_pool.tile([P, KO, 512], f32r, tag="ktin", name="ktin")
                    for eo in range(KO):
                        nc.sync.dma_start(
                            kt_in[:, eo, :], kt_dram[:, eo, kc * 512 : (kc + 1) * 512]
                        )
                    for ks in range(4):
                        pa = psA_p.tile([P, 512], f32, tag="psA", name="pas")
                        for eo in range(KO):
                            nc.tensor.matmul(
                                pa[:],
                                kt_in[:, eo, ks * P : (ks + 1) * P],
                                q_sl[:, eo, :],
                                start=(eo == 0), stop=(eo == KO - 1),
                            )
                        nc.scalar.activation(
                            E[:, kc * 4 + ks, :], pa[:], AF.Exp, scale=inv_sqrt_dk
                        )
                for qs in range(4):
                    pb = psB_p.tile([P, 512], f32, tag="psB", name="pbav")
                    pc = psC_p.tile([P, 512], f32, tag="psC", name="pcav")
                    pr = psR_p.tile([P, 2], f32, tag="psR", name="prav")
                    for ko in range(S // P):
                        lh = E[:, ko, qs * P : (qs + 1) * P]
                        nc.tensor.matmul(
                            pb[:], lh, v[:, ko, 0:512],
                            start=(ko == 0), stop=(ko == S // P - 1),
                        )
                        nc.tensor.matmul(
                            pc[:], lh, v[:, ko, 512:1024],
                            start=(ko == 0), stop=(ko == S // P - 1),
                        )
                        nc.tensor.matmul(
                            pr[:], lh, ones_t[:],
                            start=(ko == 0), stop=(ko == S // P - 1),
                        )
                    recip = msc_pool.tile([P, 1], f32, tag="recip", name="recip")
                    nc.vector.reciprocal(recip[:], pr[:, 0:1])
                    row0 = qc * 512 + qs * P
                    for half, ps in ((0, pb), (1, pc)):
                        o = out_pool.tile([P, 512], f32, tag="ost", name="ost")
                        nc.vector.tensor_scalar_mul(o[:], ps[:], recip[:])
                        nc.vector.tensor_add(
                            o[:], o[:], bvb_t[:, half * 512 : (half + 1) * 512]
                        )
                        nc.gpsimd.dma_start(
                            out[row0 : row0 + P, half * 512 : (half + 1) * 512], o[:]
                        )

    nc.finalize()
    return nc


def make_in_maps(x, Wq, bq, Wk, bk, Wv, bv):
    """Build the 8 per-core input maps from full inputs."""
    x = np.asarray(x, dtype=np.float32)
    wqT = np.ascontiguousarray(np.asarray(Wq, np.float32).T)
    wkT = np.ascontiguousarray(np.asarray(Wk, np.float32).T)
    wvT = np.ascontiguousarray(np.asarray(Wv, np.float32).T)
    bqp = np.ascontiguousarray(np.asarray(bq, np.float32).reshape(KO, P).T)
    bkp = np.ascontiguousarray(np.asarray(bk, np.float32).reshape(KO, P).T)
    bvb = np.ascontiguousarray(
        np.broadcast_to(np.asarray(bv, np.float32), (P, D))
    )
    ones_np = np.ones((P, 2), np.float32)
    xT_b = [np.ascontiguousarray(x[b].T) for b in range(B)]
    in_maps = []
    for c in range(N_CORES):
        b, h = c // 2, c % 2
        in_maps.append(
            {
                "xT": xT_b[b],
                "xTq": np.ascontiguousarray(x[b, h * SQ : (h + 1) * SQ].T),
                "wqT": wqT,
                "wkT": wkT,
                "wvT": wvT,
                "bqp": bqp,
                "bkp": bkp,
                "bvb": bvb,
                "ones": ones_np,
            }
        )
    return in_maps


_NC_CACHE = None


def get_nc():
    global _NC_CACHE
    if _NC_CACHE is None:
        _NC_CACHE = build_bass()
    return _NC_CACHE


def kernel(x, Wq, bq, Wk, bk, Wv, bv, **run_kwargs):
    from concourse.bass_utils import run_bass_kernel_spmd

    nc = get_nc()
    in_maps = make_in_maps(x, Wq, bq, Wk, bk, Wv, bv)
    res = run_bass_kernel_spmd(
        nc, in_maps, core_ids=list(range(N_CORES)), **run_kwargs
    )
    out = np.empty((B, S, D), dtype=np.float32)
    for c in range(N_CORES):
        b, h = c // 2, c % 2
        out[b, h * SQ : (h + 1) * SQ, :] = res.results[c]["out"]
    if run_kwargs.get("trace"):
        kernel.last_results = res
    return out


# revision 9
# speedup vs baseline: 1.1357x; 1.0649x over previous
"""Full-width attention (B=4, S=2048, D=1024, no head split) on 8 TRN2 cores.

Sharding: data-parallel over (batch, query-half) -> 8 shards. Core c handles
batch b = c//2, query rows [h*1024, (h+1)*1024) with h = c%2. Each core
computes K/V projections for its full batch (redundantly with its pair core),
Q projection for its query half, then scores^T -> exp -> AV locally.

Layout trick: everything is computed without any on-device transposes.
  - host passes x^T (d-major) per batch, plus W^T for each projection
  - Q^T[e,s] = (Wq^T)^T.T @ x^T   (lhsT=WqT, rhs=xT)  -> e on partitions
  - K^T[e,s] likewise, staged to DRAM scratch and re-streamed
  - V[s,e]   = (x^T).T @ Wv^T     (lhsT=xT,  rhs=WvT) -> s on partitions
  - scores^T[k,q] = KT.T @ QT (contract e)            -> k on partitions
  - softmax without max-subtraction (|scores| <= ~25, exp is safe in fp32):
    E = exp(scores^T / 8); rowsum via matmul with ones-vector rhs;
    out[q,e] = E.T @ V (contract k), scaled by 1/rowsum per partition.
  - bv folded in at the end: softmax rows sum to 1, so out += bv.
All matmuls run as float32r (1-pass FP22) at full PE speed.
"""

import math
from contextlib import ExitStack

import numpy as np

P = 128
B, S, D = 4, 2048, 1024
SQ = 1024  # query rows per core
KO = D // P  # 8 chunks of contraction dim
N_CORES = 8
F32 = None  # set after mybir import


def _apply_drain_patch():
    # This container's walrus rejects >1 sync-wait on the kernel-tail Drain
    # ("Too many sync wait commands"). Split the global-clock waits across
    # multiple drain instructions, one wait each.
    from concourse import tile as tile_mod
    from concourse.vector_clock import VectorClock, ScopedClock

    if getattr(tile_mod.TileContext, "_drain_patched", False):
        return

    def _split_drain_and_barrier(self, tick_clock, wait_clock):
        gc = tick_clock.global_clock
        n = len(gc)
        procs = [i for i in range(n) if gc[i] > 0]
        if not procs:
            d = self.nc.sync.drain()
            wait_clock.add_sem_waits(d.ins, ScopedClock({None: gc}))
        for p in procs:
            vc = VectorClock([gc[i] if i == p else 0 for i in range(n)])
            d = self.nc.sync.drain()
            wait_clock.add_sem_waits(d.ins, ScopedClock({None: vc}))
        self.nc.all_engine_barrier()
        assert self.sems is not None
        popped = self.nc._tile_sem_poison_stack.pop()
        assert popped is self._sem_poison
        self.nc.clear_and_free_semaphores(list(self.sems.allocated().values()))
        self.nc.all_engine_barrier()

    tile_mod.TileContext._drain_and_barrier = _split_drain_and_barrier
    tile_mod.TileContext._drain_patched = True


def build_bass():
    from concourse import bacc
    import concourse.mybir as mybir
    from concourse.tile import TileContext

    f32 = mybir.dt.float32
    f32r = mybir.dt.float32r
    AF = mybir.ActivationFunctionType

    nc = bacc.Bacc(
        "TRN2",
        target_bir_lowering=False,
        debug=False,
        enable_asserts=False,
        num_devices=N_CORES,
    )

    xT = nc.dram_tensor("xT", [D, S], f32r, kind="ExternalInput")
    xTq = nc.dram_tensor("xTq", [D, SQ], f32r, kind="ExternalInput")
    wqT = nc.dram_tensor("wqT", [D, D], f32r, kind="ExternalInput")
    wkT = nc.dram_tensor("wkT", [D, D], f32r, kind="ExternalInput")
    wvT = nc.dram_tensor("wvT", [D, D], f32r, kind="ExternalInput")
    bqp = nc.dram_tensor("bqp", [P, KO], f32, kind="ExternalInput")
    bkp = nc.dram_tensor("bkp", [P, KO], f32, kind="ExternalInput")
    bvb = nc.dram_tensor("bvb", [P, D], f32, kind="ExternalInput")
    ones = nc.dram_tensor("ones", [P, 2], f32r, kind="ExternalInput")
    out = nc.dram_tensor("out", [SQ, D], f32, kind="ExternalOutput")

    xT_r = xT[:, :].rearrange("(ko p) s -> p ko s", p=P)
    xTq_r = xTq[:, :].rearrange("(ko p) s -> p ko s", p=P)
    wqT_r = wqT[:, :].rearrange("(ko p) e -> p ko e", p=P)
    wkT_r = wkT[:, :].rearrange("(ko p) e -> p ko e", p=P)
    wvT_r = wvT[:, :].rearrange("(ko p) e -> p ko e", p=P)

    with TileContext(nc) as tc, ExitStack() as ctx:
        qt_pool = ctx.enter_context(tc.tile_pool(name="qtp", bufs=1))
        v_pool = ctx.enter_context(tc.tile_pool(name="vp", bufs=1))
        cpool = ctx.enter_context(tc.tile_pool(name="cp", bufs=1))
        psA_p = ctx.enter_context(tc.tile_pool(name="psA", bufs=3, space="PSUM"))
        psB_p = ctx.enter_context(tc.tile_pool(name="psB", bufs=2, space="PSUM"))
        psC_p = ctx.enter_context(tc.tile_pool(name="psC", bufs=2, space="PSUM"))
        psR_p = ctx.enter_context(tc.tile_pool(name="psR", bufs=1, space="PSUM"))
        dram_p = ctx.enter_context(tc.tile_pool(name="drp", bufs=1, space="DRAM"))

        qt = qt_pool.tile([P, KO, SQ], f32r)  # Q^T, e on partitions
        v = v_pool.tile([P, S // P, D], f32r)  # V, s on partitions
        kt_dram = dram_p.tile([P, KO, S], f32r)  # K^T staging

        bqp_t = cpool.tile([P, KO], f32)
        nc.gpsimd.dma_start(bqp_t[:], bqp[:, :])
        bkp_t = cpool.tile([P, KO], f32)
        nc.gpsimd.dma_start(bkp_t[:], bkp[:, :])
        bvb_t = cpool.tile([P, D], f32)
        nc.gpsimd.dma_start(bvb_t[:], bvb[:, :])
        ones_t = cpool.tile([P, 2], f32r)
        nc.gpsimd.dma_start(ones_t[:], ones[:, :])

        # ---------------- Phase 1: projections ----------------
        with (
            tc.tile_pool(name="xtp", bufs=3) as xt_pool,
            tc.tile_pool(name="wp", bufs=3) as w_pool,
            tc.tile_pool(name="kst", bufs=3) as kst_pool,
        ):
            # --- V = x @ Wv^T ---
            wv = [
                w_pool.tile([P, KO, 512], f32r, tag="w", name=f"wv{half}")
                for half in range(2)
            ]
            xtv0 = xt_pool.tile([P, KO, 512], f32r, tag="xt", name="xtv0")
            for ko in range(KO):
                nc.sync.dma_start(wv[0][:, ko, :], wvT_r[:, ko, 0:512])
                nc.sync.dma_start(wv[1][:, ko, :], wvT_r[:, ko, 512:1024])
                nc.sync.dma_start(xtv0[:, ko, :], xT_r[:, ko, 0:512])
            for sc in range(4):
                if sc == 0:
                    xt_c = xtv0
                else:
                    xt_c = xt_pool.tile([P, KO, 512], f32r, tag="xt", name=f"xtv{sc}")
                    for ko in range(KO):
                        nc.sync.dma_start(
                            xt_c[:, ko, :], xT_r[:, ko, sc * 512 : (sc + 1) * 512]
                        )
                for ss in range(4):
                    si = sc * 4 + ss
                    pb = psB_p.tile([P, 512], f32, tag="psB", name="pbv")
                    pc = psC_p.tile([P, 512], f32, tag="psC", name="pcv")
                    for ko in range(KO):
                        lh = xt_c[:, ko, ss * P : (ss + 1) * P]
                        nc.tensor.matmul(
                            pb[:], lh, wv[0][:, ko, :],
                            start=(ko == 0), stop=(ko == KO - 1),
                        )
                        nc.tensor.matmul(
                            pc[:], lh, wv[1][:, ko, :],
                            start=(ko == 0), stop=(ko == KO - 1),
                        )
                    nc.scalar.copy(v[:, si, 0:512], pb[:])
                    nc.scalar.copy(v[:, si, 512:1024], pc[:])

            # --- K^T = Wk @ x^T (+bk), staged to DRAM ---
            wk = []
            for half in range(2):
                wt = w_pool.tile([P, KO, 512], f32r, tag="w", name=f"wk{half}")
                nc.sync.dma_start(wt[:], wkT_r[:, :, half * 512 : (half + 1) * 512])
                wk.append(wt)
            for sc in range(4):
                xt_c = xt_pool.tile([P, KO, 512], f32r, tag="xt", name=f"xtk{sc}")
                for ko in range(KO):
                    nc.sync.dma_start(
                        xt_c[:, ko, :], xT_r[:, ko, sc * 512 : (sc + 1) * 512]
                    )
                for eo in range(KO):
                    pa = psA_p.tile([P, 512], f32, tag="psA", name="pak")
                    wkh = wk[eo // 4]
                    col = (eo % 4) * P
                    for ko in range(KO):
                        nc.tensor.matmul(
                            pa[:], wkh[:, ko, col : col + P], xt_c[:, ko, :],
                            start=(ko == 0), stop=(ko == KO - 1),
                        )
                    kst = kst_pool.tile([P, 512], f32r, tag="kst", name="kst")
                    nc.scalar.activation(kst[:], pa[:], AF.Identity, bias=bkp_t[:, eo : eo + 1])
                    nc.gpsimd.dma_start(kt_dram[:, eo, sc * 512 : (sc + 1) * 512], kst[:])

            # --- Q^T = Wq @ xq^T (+bq), kept resident ---
            wq = []
            for half in range(2):
                wt = w_pool.tile([P, KO, 512], f32r, tag="w", name=f"wq{half}")
                nc.sync.dma_start(wt[:], wqT_r[:, :, half * 512 : (half + 1) * 512])
                wq.append(wt)
            for qc2 in range(2):
                xt_c = xt_pool.tile([P, KO, 512], f32r, tag="xt", name=f"xtq{qc2}")
                for ko in range(KO):
                    nc.sync.dma_start(
                        xt_c[:, ko, :], xTq_r[:, ko, qc2 * 512 : (qc2 + 1) * 512]
                    )
                for eo in range(KO):
                    pa = psA_p.tile([P, 512], f32, tag="psA", name="paq")
                    wqh = wq[eo // 4]
                    col = (eo % 4) * P
                    for ko in range(KO):
                        nc.tensor.matmul(
                            pa[:], wqh[:, ko, col : col + P], xt_c[:, ko, :],
                            start=(ko == 0), stop=(ko == KO - 1),
                        )
                    nc.scalar.activation(
                        qt[:, eo, qc2 * 512 : (qc2 + 1) * 512],
                        pa[:],
                        AF.Identity,
                        bias=bqp_t[:, eo : eo + 1],
                    )

        # ---------------- Phase 2: attention ----------------
        with (
            tc.tile_pool(name="ep", bufs=1) as e_pool,
            tc.tile_pool(name="ktp", bufs=2) as kt_pool,
            tc.tile_pool(name="osp", bufs=3) as out_pool,
            tc.tile_pool(name="msc", bufs=4) as msc_pool,
        ):
            inv_sqrt_dk = 1.0 / math.sqrt(D // 16)  # d_key = 64
            for qc in range(2):
                E = e_pool.tile([P, S // P, 512], f32r, tag="E", name="E")
                q_sl = qt[:, :, qc * 512 : (qc + 1) * 512]
                pr = psR_p.tile([1, 512], f32, tag="psR", name="pr")
                for kc in range(4):
                    kt_in = kt_pool.tile([P, KO, 512], f32r, tag="ktin", name="ktin")
                    for eo in range(KO):
                        nc.sync.dma_start(
                            kt_in[:, eo, :], kt_dram[:, eo, kc * 512 : (kc + 1) * 512]
                        )
                    for ks in range(4):
                        idx = kc * 4 + ks
                        pa = psA_p.tile([P, 512], f32, tag="psA", name="pas")
                        for eo in range(KO):
                            nc.tensor.matmul(
                                pa[:],
                                kt_in[:, eo, ks * P : (ks + 1) * P],
                                q_sl[:, eo, :],
                                start=(eo == 0), stop=(eo == KO - 1),
                            )
                        nc.scalar.activation(
                            E[:, idx, :], pa[:], AF.Exp, scale=inv_sqrt_dk
                        )
                        nc.tensor.matmul(
                            pr[:],
                            ones_t[:, 0:1],
                            E[:, idx, :],
                            start=(idx == 0), stop=(idx == 15),
                        )
                # rowsum [1,512] -> per-partition recips [128,4]
                rsum_row = msc_pool.tile([1, 512], f32, tag="rsr", name="rsum_row")
                nc.scalar.copy(rsum_row[:], pr[:])
                rs_dram = dram_p.tile([1, 512], f32, tag="rsd", name="rs_dram")
                nc.sync.dma_start(rs_dram[:, :], rsum_row[:, :])
                rsum_t = msc_pool.tile([P, 4], f32, tag="rst", name="rsum_t")
                nc.sync.dma_start(
                    rsum_t[:, :],
                    rs_dram[0, :].rearrange("(qs p) -> p qs", p=P),
                )
                recip = msc_pool.tile([P, 4], f32, tag="recip", name="recip")
                nc.vector.reciprocal(recip[:], rsum_t[:])
                for qs in range(4):
                    pb = psB_p.tile([P, 512], f32, tag="psB", name="pbav")
                    pc = psC_p.tile([P, 512], f32, tag="psC", name="pcav")
                    for ko in range(S // P):
                        lh = E[:, ko, qs * P : (qs + 1) * P]
                        nc.tensor.matmul(
                            pb[:], lh, v[:, ko, 0:512],
                            start=(ko == 0), stop=(ko == S // P - 1),
                        )
                        nc.tensor.matmul(
                            pc[:], lh, v[:, ko, 512:1024],
                            start=(ko == 0), stop=(ko == S // P - 1),
                        )
                    row0 = qc * 512 + qs * P
                    for half, ps in ((0, pb), (1, pc)):
                        o = out_pool.tile([P, 512], f32, tag="ost", name="ost")
                        nc.vector.tensor_scalar_mul(o[:], ps[:], recip[:, qs : qs + 1])
                        nc.vector.tensor_add(
                            o[:], o[:], bvb_t[:, half * 512 : (half + 1) * 512]
                        )
                        nc.sync.dma_start(
                            out[row0 : row0 + P, half * 512 : (half + 1) * 512], o[:]
                        )

    nc.finalize()
    return nc


def make_in_maps(x, Wq, bq, Wk, bk, Wv, bv):
    """Build the 8 per-core input maps from full inputs."""
    x = np.asarray(x, dtype=np.float32)
    wqT = np.ascontiguousarray(np.asarray(Wq, np.float32).T)
    wkT = np.ascontiguousarray(np.asarray(Wk, np.float32).T)
    wvT = np.ascontiguousarray(np.asarray(Wv, np.float32).T)
    bqp = np.ascontiguousarray(np.asarray(bq, np.float32).reshape(KO, P).T)
    bkp = np.ascontiguousarray(np.asarray(bk, np.float32).reshape(KO, P).T)
    bvb = np.ascontiguousarray(
        np.broadcast_to(np.asarray(bv, np.float32), (P, D))
    )
    ones_np = np.ones((P, 2), np.float32)
    xT_b = [np.ascontiguousarray(x[b].T) for b in range(B)]
    in_maps = []
    for c in range(N_CORES):
        b, h = c // 2, c % 2
        in_maps.append(
            {
                "xT": xT_b[b],
                "xTq": np.ascontiguousarray(x[b, h * SQ : (h + 1) * SQ].T),
                "wqT": wqT,
                "wkT": wkT,
                "wvT": wvT,
                "bqp": bqp,
                "bkp": bkp,
                "bvb": bvb,
                "ones": ones_np,
            }
        )
    return in_maps


_NC_CACHE = None


def get_nc():
    global _NC_CACHE
    if _NC_CACHE is None:
        _NC_CACHE = build_bass()
    return _NC_CACHE


def kernel(x, Wq, bq, Wk, bk, Wv, bv, **run_kwargs):
    from concourse.bass_utils import run_bass_kernel_spmd

    nc = get_nc()
    in_maps = make_in_maps(x, Wq, bq, Wk, bk, Wv, bv)
    res = run_bass_kernel_spmd(
        nc, in_maps, core_ids=list(range(N_CORES)), **run_kwargs
    )
    out = np.empty((B, S, D), dtype=np.float32)
    for c in range(N_CORES):
        b, h = c // 2, c % 2
        out[b, h * SQ : (h + 1) * SQ, :] = res.results[c]["out"]
    if run_kwargs.get("trace"):
        kernel.last_results = res
    return out
